# revision 1
# baseline (speedup 1.0000x reference)
"""Trainium2 Bass kernel for Conv2D (1x1) multi-head attention block.

Reference computation (per batch image of [64, 64, 512] = [N=4096, C=512]):
    x  = GroupNorm(inputs, G=32, eps=1e-6) * gamma + beta
    q, k, v = x @ wq + bq, x @ wk + bk, x @ wv + bv      (1x1 convs)
    scores  = (q / sqrt(C)) @ k^T                         [N, N]
    out     = softmax(scores) @ v @ wo + bo + inputs

Sharding: 8 cores = 2 batches x 4 query-quarters. Each core holds the full
image of its batch (needed for GroupNorm stats and full-attention K/V) and
computes the output rows of its query quarter only.  No collectives: the
redundant K/V compute is cheaper than a DRAM-bounce AllGather here.

Key implementation choices:
  - GroupNorm is folded into the projection weights: with per-channel
    a[c] = gamma*rstd, b[c] = beta - mean*gamma*rstd, we have
    K^T = (diag(a) wk)^T x^T + (wk^T b + bk) 1^T, so normalized
    activations are never materialized.  Stats come from ones-matmuls
    (per-channel sum / sum-of-squares) in float32r (TF32-like) during the
    single streaming pass over x.
  - x^T is produced once by PE transposes of 128x128 blocks and kept
    resident in bf16; K^T, Q^T, V and all attention matmuls run in bf16
    (fp32 PSUM accumulation).  bf16 weights get fast-weight-load, which
    roughly halves the per-matmul cost vs 4-byte dtypes.  The residual add
    and all softmax normalization stay fp32, and the attention output is
    only ~4% of the output magnitude, so end-to-end error stays ~2e-4.
  - Attention uses the transposed-scores layout: scores^T[k, q] tiles come
    from matmul(lhsT=K^T tile, rhs=Q^T chunk); exp runs on the scalar
    engine PSUM->SBUF (no max-subtraction: scores are O(1) by construction
    since q is pre-scaled by 1/sqrt(C)); probs^T feeds
    matmul(lhsT=V tile, rhs=probs^T) accumulating attn^T[c, q] in PSUM over
    all 32 key tiles, and a ones-column matmul accumulates the softmax
    denominators.  The output projection consumes the *unnormalized*
    attn^T immediately; 1/rowsum is applied per-partition at the final
    PSUM->SBUF copy, keeping the PE free of the softmax epilogue.
"""

import sys

sys.path.insert(0, "/opt/trn_rl_repo")

from contextlib import ExitStack

import numpy as np

import concourse.bacc as bacc
import concourse.tile as tile
from concourse import mybir
from concourse.bass_utils import run_bass_kernel_spmd

# Problem shape (hardcoded; kernel.py must be self-contained).
B, HH, WW, C = 2, 64, 64, 512
N = HH * WW          # 4096 pixels per batch image
G = 32               # groupnorm groups
GS = C // G          # 16 channels per group
EPS = 1e-6
P = 128              # partitions
CT = C // P          # 4 channel tiles
NT = N // P          # 32 pixel tiles per image
CHUNK = 512          # free-dim chunk for moving operands
NCH = N // CHUNK     # 8 pixel chunks per image
NCORES = 8
QS = N // 4          # 1024 query rows per core
QTILES = QS // P     # 8 query tiles per core
QCH = QS // CHUNK    # 2 query chunks per core
GROUP_COUNT = N * GS  # elements per (batch, group) for the mean/var

F32 = mybir.dt.float32
F32R = mybir.dt.float32r
BF16 = mybir.dt.bfloat16
AF = mybir.ActivationFunctionType

_NC_CACHE = None


def _build():
    nc = bacc.Bacc(None, target_bir_lowering=False, debug=False)

    # x arrives pre-cast to bf16 (host-side layout prep, like the
    # per-core sharding); x^T is built by hardware DMA-transpose reads
    # straight from DRAM, so the PE does no transposes at all.  The fp32
    # copy of the query quarter feeds the residual add.
    x_bf = nc.dram_tensor("x_bf", [N, C], BF16, kind="ExternalInput")
    x_resq_bf = nc.dram_tensor("x_resq_bf", [QS, C], BF16, kind="ExternalInput")
    x_res = nc.dram_tensor("x_res", [QS, C], F32, kind="ExternalInput")
    gamma_d = nc.dram_tensor("gamma", [C], F32, kind="ExternalInput")
    beta_d = nc.dram_tensor("beta", [C], F32, kind="ExternalInput")
    w_d = {}
    b_d = {}
    for nm in ("wq", "wk", "wv", "wo"):
        w_d[nm] = nc.dram_tensor(nm, [C, C], F32, kind="ExternalInput")
    for nm in ("bq", "bk", "bv", "bo"):
        b_d[nm] = nc.dram_tensor(nm, [C], F32, kind="ExternalInput")
    ident_d = nc.dram_tensor("ident", [P, P], F32R, kind="ExternalInput")
    gind_d = nc.dram_tensor("gind", [P, 8], F32, kind="ExternalInput")
    gindt_d = nc.dram_tensor("gindt", [8, P], F32, kind="ExternalInput")
    out_d = nc.dram_tensor("out", [QS, C], F32, kind="ExternalOutput")

    with tile.TileContext(nc) as tc, ExitStack() as top:
        # ---- persistent pools ----
        consts = top.enter_context(tc.tile_pool(name="consts", bufs=1))
        pkt = top.enter_context(tc.tile_pool(name="pkt", bufs=1))
        pqt = top.enter_context(tc.tile_pool(name="pqt", bufs=1))
        pv = top.enter_context(tc.tile_pool(name="pv", bufs=1))
        pxt = top.enter_context(tc.tile_pool(name="pxt", bufs=1))
        pmisc = top.enter_context(tc.tile_pool(name="pmisc", bufs=1))

        ident = consts.tile([P, P], F32R, name="ident")
        nc.sync.dma_start(out=ident, in_=ident_d[:])
        gind = consts.tile([P, 8], F32, name="gind")
        nc.sync.dma_start(out=gind, in_=gind_d[:])
        gindt = consts.tile([8, P], F32, name="gindt")
        nc.sync.dma_start(out=gindt, in_=gindt_d[:])
        ones_f32 = consts.tile([P, 1], F32, name="ones_f32")
        nc.vector.memset(ones_f32, 1.0)
        ones_bf = consts.tile([P, 1], BF16, name="ones_bf")
        nc.scalar.copy(ones_bf, ones_f32)
        one11 = ones_f32[0:1, 0:1]

        gamma4, beta4 = [], []
        for ct in range(CT):
            gt_ = consts.tile([P, 1], F32, name=f"gamma4_{ct}")
            nc.sync.dma_start(out=gt_, in_=gamma_d[ct * P:(ct + 1) * P])
            gamma4.append(gt_)
            bt_ = consts.tile([P, 1], F32, name=f"beta4_{ct}")
            nc.sync.dma_start(out=bt_, in_=beta_d[ct * P:(ct + 1) * P])
            beta4.append(bt_)

        # Resident activations: x^T, K^T, Q^T, V natural -- all bf16
        xt = [pxt.tile([P, N], BF16, name=f"xt{i}", tag=f"xt{i}") for i in range(CT)]
        kt = [pkt.tile([P, N], BF16, name=f"kt{i}", tag=f"kt{i}") for i in range(CT)]
        qt = [pqt.tile([P, QS], BF16, name=f"qt{i}", tag=f"qt{i}") for i in range(CT)]
        vv = [pv.tile([P, C], BF16, name=f"v{i}", tag=f"v{i}") for i in range(NT)]
        # x^T of the query quarter (for Q projection)
        xtq = [pxt.tile([P, QS], BF16, name=f"xtq{i}", tag=f"xtq{i}")
               for i in range(CT)]

        with ExitStack() as dphase:
            psp = dphase.enter_context(tc.tile_pool(name="psp", bufs=3, space="PSUM"))

            # per-channel bn_stats accumulators, one [P, NCH, 6] per ct
            bnst = [pmisc.tile([P, NCH, 6], F32, name=f"bnst{i}")
                    for i in range(CT)]

            # Warm-keeper: idle-PE filler matmuls so the HAM clock gate
            # stays at full rate while DMA/DVE do the x^T build.
            warm32 = pmisc.tile([P, CHUNK], F32, name="warm32")
            nc.vector.memset(warm32, 1.0)
            warm_src = pmisc.tile([P, CHUNK], F32R, name="warm_src")
            nc.scalar.copy(warm_src, warm32)

            def keep_warm(n):
                for _ in range(n):
                    wps = psp.tile([P, CHUNK], F32, name="wps", tag="kps")
                    nc.tensor.matmul(wps, lhsT=ident, rhs=warm_src,
                                     start=True, stop=True)

            # ==== Phase A: build x^T via hardware DMA-transpose ====
            for ch in range(NCH):
                for ct in range(CT):
                    nc.sync.dma_start_transpose(
                        xt[ct][:, ch * CHUNK:(ch + 1) * CHUNK],
                        x_bf[ch * CHUNK:(ch + 1) * CHUNK, ct * P:(ct + 1) * P])
                    nc.vector.bn_stats(
                        out=bnst[ct][:, ch, :],
                        in_=xt[ct][:, ch * CHUNK:(ch + 1) * CHUNK])
                keep_warm(3)
            for ch in range(QCH):
                for ct in range(CT):
                    nc.sync.dma_start_transpose(
                        xtq[ct][:, ch * CHUNK:(ch + 1) * CHUNK],
                        x_resq_bf[ch * CHUNK:(ch + 1) * CHUNK,
                                  ct * P:(ct + 1) * P])
                keep_warm(3)

            # ==== Phase B: group stats -> per-channel a, b (partition-major)
            a4, aq4, b4 = [], [], []
            with tc.tile_pool(name="psb", bufs=1, space="PSUM") as psb, \
                 tc.tile_pool(name="pb", bufs=2) as pb:
                for ct in range(CT):
                    mv = pb.tile([P, 2], F32, name="mv", tag="mv")
                    nc.vector.bn_aggr(out=mv, in_=bnst[ct])
                    # per-channel (mean, E[x^2])
                    me2 = pb.tile([P, 2], F32, name="me2", tag="me2")
                    nc.vector.tensor_copy(me2[:, 0:1], mv[:, 0:1])
                    nc.vector.tensor_mul(me2[:, 1:2], mv[:, 0:1], mv[:, 0:1])
                    nc.vector.tensor_add(me2[:, 1:2], me2[:, 1:2], mv[:, 1:2])
                    keep_warm(6)
                    grp_ps = psb.tile([8, 2], F32, name="grp_ps", tag="grp_ps")
                    nc.tensor.matmul(grp_ps, lhsT=gind, rhs=me2,
                                     start=True, stop=True)
                    grp = pb.tile([8, 2], F32, name="grp", tag="grp")
                    nc.vector.tensor_scalar_mul(grp, grp_ps, 1.0 / GS)
                    var = pb.tile([8, 1], F32, name="var", tag="var")
                    nc.vector.tensor_mul(var, grp[:, 0:1], grp[:, 0:1])
                    nc.vector.tensor_sub(var, grp[:, 1:2], var)
                    nc.vector.tensor_scalar_add(var, var, EPS)
                    rstd = pb.tile([8, 1], F32, name="rstd", tag="rstd")
                    nc.vector.reciprocal(rstd, var)
                    nc.scalar.sqrt(rstd, rstd)
                    mr = pb.tile([8, 2], F32, name="mr", tag="mr")
                    nc.vector.tensor_copy(mr[:, 0:1], grp[:, 0:1])
                    nc.vector.tensor_copy(mr[:, 1:2], rstd)
                    mch_ps = psb.tile([P, 2], F32, name="mch_ps", tag="mch_ps")
                    nc.tensor.matmul(mch_ps, lhsT=gindt, rhs=mr,
                                     start=True, stop=True)
                    keep_warm(6)
                    mch = pb.tile([P, 2], F32, name="mch", tag="mch")
                    nc.vector.tensor_copy(mch, mch_ps)
                    a_t = pmisc.tile([P, 1], F32, name=f"a4_{ct}")
                    nc.vector.tensor_mul(a_t, gamma4[ct], mch[:, 1:2])
                    a4.append(a_t)
                    aq_t = pmisc.tile([P, 1], F32, name=f"aq4_{ct}")
                    nc.vector.tensor_scalar_mul(aq_t, a_t, 1.0 / float(np.sqrt(C)))
                    aq4.append(aq_t)
                    b_t = pmisc.tile([P, 1], F32, name=f"b4_{ct}")
                    nc.vector.tensor_mul(b_t, mch[:, 0:1], a_t)
                    nc.vector.tensor_sub(b_t, beta4[ct], b_t)
                    b4.append(b_t)

            # ==== Phase C: fold weights (bf16) + biases ====
            def fold_weight(nm, scales, qscale, pool, pspool, wpool):
                wf, raws = [], []
                for ct in range(CT):
                    raw = wpool.tile([P, C], F32, name=f"{nm}_raw",
                                     tag=f"{nm}_raw")
                    nc.sync.dma_start(out=raw,
                                      in_=w_d[nm][ct * P:(ct + 1) * P, :])
                    raws.append(raw)
                    wf_t = pool.tile([P, C], BF16, name=f"{nm}_f{ct}",
                                     tag=f"{nm}_f{ct}")
                    nc.scalar.mul(wf_t, raw, scales[ct])
                    wf.append(wf_t)
                keep_warm(8)
                bias_ps = pspool.tile([1, C], F32, name=f"{nm}_bps", tag="bias")
                for ct in range(CT):
                    nc.tensor.matmul(bias_ps, lhsT=b4[ct], rhs=raws[ct],
                                     start=(ct == 0), stop=(ct == CT - 1))
                bnm = "b" + nm[1:]
                braw = wpool.tile([1, C], F32, name=f"{bnm}_raw", tag="braw")
                nc.sync.dma_start(out=braw, in_=b_d[bnm][:])
                bias_sb = pmisc.tile([1, C], F32, name=f"{bnm}_sb")
                nc.vector.tensor_add(bias_sb, bias_ps, braw)
                if qscale is not None:
                    nc.vector.tensor_scalar_mul(bias_sb, bias_sb, qscale)
                keep_warm(4)
                b_pm = []
                for ct in range(CT):
                    bp_ps = pspool.tile([P, 1], F32, name=f"{bnm}_pps",
                                        tag="bias")
                    nc.tensor.matmul(bp_ps,
                                     lhsT=bias_sb[0:1, ct * P:(ct + 1) * P],
                                     rhs=one11, start=True, stop=True)
                    bp = pmisc.tile([P, 1], F32, name=f"{bnm}4_{ct}")
                    nc.vector.tensor_copy(bp, bp_ps)
                    b_pm.append(bp)
                return wf, bias_sb, b_pm

            with tc.tile_pool(name="pw", bufs=1) as pw, \
                 tc.tile_pool(name="pwraw", bufs=1) as pwraw, \
                 tc.tile_pool(name="psc", bufs=2, space="PSUM") as psc:
                wk_f, _, bk4 = fold_weight("wk", a4, None, pw, psc, pwraw)
                wq_f, _, bq4 = fold_weight(
                    "wq", aq4, 1.0 / float(np.sqrt(C)), pw, psc, pwraw)
                wv_f, bv_sb, _ = fold_weight("wv", a4, None, pw, psc, pwraw)
                bv_b = pmisc.tile([P, C], F32, name="bv_b")
                nc.gpsimd.partition_broadcast(bv_b, bv_sb)

                # ==== Phase D: projections from resident x^T ====
                # K^T[co][:, chunk] = sum_ct wk'[ct][:,co*128:] ^T @ x^T[ct]
                for ch in range(NCH):
                    for co in range(CT):
                        kps = psp.tile([P, CHUNK], F32, name="kps", tag="kps")
                        for ct in range(CT):
                            nc.tensor.matmul(
                                kps, lhsT=wk_f[ct][:, co * P:(co + 1) * P],
                                rhs=xt[ct][:, ch * CHUNK:(ch + 1) * CHUNK],
                                start=(ct == 0), stop=(ct == CT - 1))
                        nc.scalar.activation(
                            kt[co][:, ch * CHUNK:(ch + 1) * CHUNK], kps,
                            AF.Identity, bias=bk4[co], scale=1.0)
                for ch in range(QCH):
                    for co in range(CT):
                        qps = psp.tile([P, CHUNK], F32, name="qps", tag="kps")
                        for ct in range(CT):
                            nc.tensor.matmul(
                                qps, lhsT=wq_f[ct][:, co * P:(co + 1) * P],
                                rhs=xtq[ct][:, ch * CHUNK:(ch + 1) * CHUNK],
                                start=(ct == 0), stop=(ct == CT - 1))
                        nc.scalar.activation(
                            qt[co][:, ch * CHUNK:(ch + 1) * CHUNK], qps,
                            AF.Identity, bias=bq4[co], scale=1.0)
                # V natural: lhsT = x^T pixel-block, rhs = wv'
                for nt_i in range(NT):
                    vps = psp.tile([P, C], F32, name="vps", tag="kps")
                    for ct in range(CT):
                        nc.tensor.matmul(
                            vps, lhsT=xt[ct][:, nt_i * P:(nt_i + 1) * P],
                            rhs=wv_f[ct], start=(ct == 0), stop=(ct == CT - 1))
                    nc.vector.tensor_add(vv[nt_i], vps, bv_b)

        # ==== Phase E/F: attention + output projection ====
        with tc.tile_pool(name="pwo", bufs=1) as pwo, \
             tc.tile_pool(name="pres", bufs=1) as pres, \
             tc.tile_pool(name="pe", bufs=3) as pe, \
             tc.tile_pool(name="pef", bufs=2) as pef, \
             tc.tile_pool(name="pss", bufs=2, space="PSUM") as pss, \
             tc.tile_pool(name="psat", bufs=1, space="PSUM") as psat, \
             tc.tile_pool(name="psr", bufs=1, space="PSUM") as psr, \
             tc.tile_pool(name="pso", bufs=1, space="PSUM") as pso:
            wo_f = []
            for ct in range(CT):
                raw = pef.tile([P, C], F32, name="wo_raw", tag="wo_raw")
                nc.sync.dma_start(out=raw, in_=w_d["wo"][ct * P:(ct + 1) * P, :])
                wo_t = pwo.tile([P, C], BF16, name=f"wo_f{ct}", tag=f"wo_f{ct}")
                nc.scalar.copy(wo_t, raw)
                wo_f.append(wo_t)
            bo_raw = pef.tile([1, C], F32, name="bo_raw", tag="bo_raw")
            nc.sync.dma_start(out=bo_raw, in_=b_d["bo"][:])
            bo_b = pwo.tile([P, C], F32, name="bo_b", tag="bo_b")
            nc.gpsimd.partition_broadcast(bo_b, bo_raw)
            resb = []
            for i in range(QTILES):
                rraw = pef.tile([P, C], F32, name="rraw", tag="rraw")
                nc.sync.dma_start(out=rraw, in_=x_res[i * P:(i + 1) * P, :])
                rb = pres.tile([P, C], F32, name=f"resb{i}", tag=f"resb{i}")
                nc.vector.tensor_add(rb, rraw, bo_b)
                resb.append(rb)

            at_ps = [psat.tile([P, CHUNK], F32, name=f"at{i}", tag=f"at{i}")
                     for i in range(CT)]
            for qc in range(QCH):
                rows_ps = psr.tile([1, CHUNK], F32, name="rows", tag="rows")
                for kt_i in range(NT):
                    sc_ps = pss.tile([P, CHUNK], F32, name="sc", tag="sc")
                    for ct in range(CT):
                        nc.tensor.matmul(
                            sc_ps,
                            lhsT=kt[ct][:, kt_i * P:(kt_i + 1) * P],
                            rhs=qt[ct][:, qc * CHUNK:(qc + 1) * CHUNK],
                            start=(ct == 0), stop=(ct == CT - 1))
                    probs = pe.tile([P, CHUNK], BF16, name="probs", tag="probs")
                    nc.scalar.activation(probs, sc_ps, AF.Exp)
                    for co in range(CT):
                        nc.tensor.matmul(
                            at_ps[co],
                            lhsT=vv[kt_i][:, co * P:(co + 1) * P],
                            rhs=probs,
                            start=(kt_i == 0), stop=(kt_i == NT - 1))
                    nc.tensor.matmul(rows_ps, lhsT=ones_bf, rhs=probs,
                                     start=(kt_i == 0), stop=(kt_i == NT - 1))
                # softmax denominators -> per-partition reciprocals
                recip = pe.tile([1, CHUNK], F32, name="recip", tag="recip")
                nc.vector.reciprocal(recip, rows_ps)
                recip4 = []
                for qi in range(4):
                    r4_ps = psr.tile([P, 1], F32, name="r4", tag="rows")
                    nc.tensor.matmul(r4_ps,
                                     lhsT=recip[0:1, qi * P:(qi + 1) * P],
                                     rhs=one11, start=True, stop=True)
                    r4 = pe.tile([P, 1], F32, name="recip4", tag=f"recip4_{qi}")
                    nc.vector.tensor_copy(r4, r4_ps)
                    recip4.append(r4)
                # unnormalized attn^T -> SBUF (no dependency on rowsums)
                at_sb = []
                for co in range(CT):
                    a_sb = pe.tile([P, CHUNK], BF16, name="at_sb",
                                   tag=f"at_sb{co}")
                    nc.scalar.copy(a_sb, at_ps[co])
                    at_sb.append(a_sb)
                for qi in range(4):
                    ops = pso.tile([P, C], F32, name="ops", tag="ops")
                    for ct in range(CT):
                        nc.tensor.matmul(
                            ops, lhsT=at_sb[ct][:, qi * P:(qi + 1) * P],
                            rhs=wo_f[ct], start=(ct == 0), stop=(ct == CT - 1))
                    fin = pe.tile([P, C], F32, name="fin", tag="fin")
                    # normalize rows here: out_row *= 1/rowsum (per-partition)
                    nc.scalar.activation(fin, ops, AF.Copy, bias=0.0,
                                         scale=recip4[qi])
                    fin2 = pe.tile([P, C], F32, name="fin2", tag="fin2")
                    nc.vector.tensor_add(fin2, fin, resb[qc * 4 + qi])
                    r0 = (qc * 4 + qi) * P
                    nc.sync.dma_start(out=out_d[r0:r0 + P, :], in_=fin2)

    nc.compile()
    return nc


def _consts():
    ident = np.eye(P, dtype=np.float32)
    gind = np.zeros((P, 8), dtype=np.float32)
    for p in range(P):
        gind[p, p // GS] = 1.0
    gindt = np.ascontiguousarray(gind.T)
    return ident, gind, gindt


def _make_in_maps(inputs):
    import ml_dtypes
    x = np.ascontiguousarray(np.asarray(inputs["inputs"], dtype=np.float32))
    xf = x.reshape(B, N, C)
    xf_bf = xf.astype(ml_dtypes.bfloat16)
    ident, gind, gindt = _consts()
    shared = {
        "gamma": np.ascontiguousarray(np.asarray(inputs["gn_gamma"], np.float32)),
        "beta": np.ascontiguousarray(np.asarray(inputs["gn_beta"], np.float32)),
        "ident": ident, "gind": gind, "gindt": gindt,
    }
    for nm in ("wq", "wk", "wv", "wo", "bq", "bk", "bv", "bo"):
        shared[nm] = np.ascontiguousarray(np.asarray(inputs[nm], np.float32))
    in_maps = []
    for core in range(NCORES):
        b, qq = divmod(core, 4)
        xr = np.ascontiguousarray(xf[b, qq * QS:(qq + 1) * QS, :])
        m = dict(shared)
        m["x_bf"] = np.ascontiguousarray(xf_bf[b])
        m["x_resq_bf"] = np.ascontiguousarray(xf_bf[b, qq * QS:(qq + 1) * QS, :])
        m["x_res"] = xr
        in_maps.append(m)
    return in_maps


def _assemble(results):
    out = np.empty((B, N, C), dtype=np.float32)
    for core in range(NCORES):
        b, qq = divmod(core, 4)
        out[b, qq * QS:(qq + 1) * QS, :] = results[core]["out"]
    return out.reshape(B, HH, WW, C)


def kernel(**inputs):
    global _NC_CACHE
    if _NC_CACHE is None:
        _NC_CACHE = _build()
    in_maps = _make_in_maps(inputs)
    res = run_bass_kernel_spmd(_NC_CACHE, in_maps, list(range(NCORES)))
    return _assemble(res.results)


def _install_ntff_shim():
    """The agent image's antenv lacks axon_hooks; provide it so
    run_bass_kernel_spmd(trace=True) can NTFF-profile through axon."""
    import types
    import antenv
    if "antenv.axon_hooks" in sys.modules:
        return
    mod = types.ModuleType("antenv.axon_hooks")
    mod._hook = None

    def set_axon_ntff_profile_hook(h):
        mod._hook = h

    def get_axon_ntff_profile_hook():
        return mod._hook

    mod.set_axon_ntff_profile_hook = set_axon_ntff_profile_hook
    mod.get_axon_ntff_profile_hook = get_axon_ntff_profile_hook
    sys.modules["antenv.axon_hooks"] = mod
    antenv.axon_hooks = mod
    sys.path.insert(0, "/root/.axon_site")
    from trn_agent_boot.trn_boot import _ntff_profile_via_ctypes
    hook = _ntff_profile_via_ctypes("/opt/axon/libaxon_pjrt.so")
    set_axon_ntff_profile_hook(hook)


def run_traced(inputs, trace_kwargs=None):
    """Traced run for profiling: returns (BassKernelResults, tmpdir)."""
    global _NC_CACHE
    if _NC_CACHE is None:
        _NC_CACHE = _build()
    import tempfile
    _install_ntff_shim()
    in_maps = _make_in_maps(inputs)
    tmpdir = tempfile.mkdtemp(prefix="trace_")
    res = run_bass_kernel_spmd(_NC_CACHE, in_maps, list(range(NCORES)),
                               trace=True, tmpdir=tmpdir,
                               trace_kwargs=trace_kwargs or {})
    return res, tmpdir



# revision 7
# speedup vs baseline: 1.8401x; 1.8401x over previous
"""Trainium2 Bass kernel for Conv2D (1x1) multi-head attention block.

Reference computation (per batch image of [64, 64, 512] = [N=4096, C=512]):
    x  = GroupNorm(inputs, G=32, eps=1e-6) * gamma + beta
    q, k, v = x @ wq + bq, x @ wk + bk, x @ wv + bv      (1x1 convs)
    scores  = (q / sqrt(C)) @ k^T                         [N, N]
    out     = softmax(scores) @ v @ wo + bo + inputs

Sharding: 8 cores = 2 batches x 4 query-quarters.  Each core holds the full
image of its batch (GroupNorm stats + full-attention K/V) and produces the
output rows of its query quarter.  No collectives.

Implementation notes:
  - The host ships x^T pre-transposed and cast to fp8_e4m3, laid out as
    channel-pair tiles [128, 2, N] so every matmul can run in fp8 DoubleRow
    perf mode (2 contraction rows per PE column cycle = 2x bf16 throughput,
    256-deep contraction per instruction).
  - GroupNorm is folded into the projection weights: a[c] = gamma*rstd,
    b[c] = beta - mean*a.  Stats come from DVE bn_stats over the resident
    fp8 x^T.  Weights arrive in bf16 and are folded to fp8 as S*a*w (S=16
    keeps fp8 values in the normal range); the 1/S is recovered in the
    PSUM->SBUF epilogues.  Projection biases (b^T w + b_orig) come from
    small bf16 GEMVs.
  - Scores are computed per 128-key tile as scores^T[k, q] (fp8 DoubleRow
    over channel pairs); exp runs on the scalar engine with scale 1/sqrt(C)
    and bias -2 (softmax shift invariance; keeps exp outputs inside fp8's
    +-240 range) writing fp8 probs pairs.  attn^T accumulates over key-pair
    tiles in PSUM; softmax denominators come from a DoubleRow ones-matmul
    into a [32, 512] PSUM tile (M=1 weight loads are ISA-illegal in dual-fp8
    mode, M=32 costs the same).  The kernel is software-pipelined: attnV of
    pair g-1 issues between the scores and exps of pair g, so the PE never
    waits on the scalar engine.
  - V's bias is NOT applied per-tile: softmax rows sum to 1, so a constant
    bias on V passes through attention unchanged and is folded into the
    output projection's bias (bo_eff = wo^T (b^T wv + bv) + bo), which is
    pre-added to the f32 residual tiles.
"""

import sys

sys.path.insert(0, "/opt/trn_rl_repo")

from contextlib import ExitStack

import numpy as np

import concourse.bacc as bacc
import concourse.tile as tile
from concourse import mybir
from concourse.bass_utils import run_bass_kernel_spmd

# Problem shape (hardcoded; kernel.py must be self-contained).
B, HH, WW, C = 2, 64, 64, 512
N = HH * WW          # 4096 pixels per image
G = 32               # groupnorm groups
GS = C // G          # 16 channels per group
EPS = 1e-6
P = 128              # partitions
CT = C // P          # 4 channel tiles
CP = CT // 2         # 2 channel-pair tiles
NT = N // P          # 32 pixel tiles per image
NP2 = NT // 2        # 16 pixel-pair tiles
NCORES = 8
QS = N // 4          # 1024 query rows per core
QTILES = QS // P     # 8 query tiles per core
QCH = QS // 512      # 2 query chunks per core

S = 16.0             # fp8 weight scale
ISQ = 1.0 / float(np.sqrt(float(C)))
SHIFT = -2.0         # exp(s*ISQ + SHIFT): keeps probs < 240 (fp8e4 max)

F32 = mybir.dt.float32
BF16 = mybir.dt.bfloat16
FP8 = mybir.dt.float8e4
AF = mybir.ActivationFunctionType
ALU = mybir.AluOpType
DR = mybir.MatmulPerfMode.DoubleRow

_NC_CACHE = None


def _build():
    nc = bacc.Bacc(None, target_bir_lowering=False, debug=False)

    xt8_d = [nc.dram_tensor(f"xt8p{g}", [P, 2, N], FP8, kind="ExternalInput")
             for g in range(CP)]
    x_res_d = nc.dram_tensor("x_res", [QS, C], F32, kind="ExternalInput")
    w16_d = {nm: nc.dram_tensor(nm, [C, C], BF16, kind="ExternalInput")
             for nm in ("wq", "wk", "wv", "wo")}
    b_d = {nm: nc.dram_tensor(nm, [C], F32, kind="ExternalInput")
           for nm in ("bq", "bk", "bv", "bo")}
    gamma_d = nc.dram_tensor("gamma", [C], F32, kind="ExternalInput")
    beta_d = nc.dram_tensor("beta", [C], F32, kind="ExternalInput")
    gind_d = nc.dram_tensor("gind", [P, 8], F32, kind="ExternalInput")
    gindt_d = nc.dram_tensor("gindt", [8, P], F32, kind="ExternalInput")
    one_d = nc.dram_tensor("one11", [1, 1], F32, kind="ExternalInput")
    out_d = nc.dram_tensor("out", [QS, C], F32, kind="ExternalOutput")

    # The same program runs on every core, so the query-quarter offset must
    # come from the data: the host ships the quarter's x^T columns as a
    # separate small input, pairs stacked along dim 1 as (g, i) -> 2g+i.
    xq8_d = nc.dram_tensor("xq8p", [P, 2 * CP, QS], FP8, kind="ExternalInput")

    with tile.TileContext(nc) as tc, ExitStack() as top:
        consts = top.enter_context(tc.tile_pool(name="consts", bufs=1))
        pxt = top.enter_context(tc.tile_pool(name="pxt", bufs=1))
        pkt = top.enter_context(tc.tile_pool(name="pkt", bufs=1))
        pqt = top.enter_context(tc.tile_pool(name="pqt", bufs=1))
        pv = top.enter_context(tc.tile_pool(name="pv", bufs=1))
        pw = top.enter_context(tc.tile_pool(name="pw", bufs=1))
        pres = top.enter_context(tc.tile_pool(name="pres", bufs=1))
        pmisc = top.enter_context(tc.tile_pool(name="pmisc", bufs=1))

        # ---------- consts ----------
        gind = consts.tile([P, 8], F32, name="gind")
        nc.sync.dma_start(out=gind, in_=gind_d[:])
        gindt = consts.tile([8, P], F32, name="gindt")
        nc.sync.dma_start(out=gindt, in_=gindt_d[:])
        one11 = consts.tile([1, 1], F32, name="one11")
        nc.sync.dma_start(out=one11, in_=one_d[:])
        ones32_8 = consts.tile([P, 2, 32], FP8, name="ones32_8")
        nc.vector.memset(ones32_8, 1.0)
        ebias = consts.tile([P, 1], F32, name="ebias")
        nc.vector.memset(ebias, SHIFT)

        gamma4, beta4 = [], []
        for ct in range(CT):
            gt_ = consts.tile([P, 1], F32, name=f"gamma4_{ct}")
            nc.sync.dma_start(out=gt_, in_=gamma_d[ct * P:(ct + 1) * P])
            gamma4.append(gt_)
            bt_ = consts.tile([P, 1], F32, name=f"beta4_{ct}")
            nc.sync.dma_start(out=bt_, in_=beta_d[ct * P:(ct + 1) * P])
            beta4.append(bt_)

        # ---------- resident tensors ----------
        xt8 = [pxt.tile([P, 2, N], FP8, name=f"xt8_{g}", tag=f"xt8_{g}")
               for g in range(CP)]
        xq8 = pxt.tile([P, 2 * CP, QS], FP8, name="xq8", tag="xq8")
        kt8 = [pkt.tile([P, 2, N], FP8, name=f"kt8_{g}", tag=f"kt8_{g}")
               for g in range(CP)]
        qt8 = [pqt.tile([P, 2, QS], FP8, name=f"qt8_{g}", tag=f"qt8_{g}")
               for g in range(CP)]
        vv8 = [pv.tile([P, 2, C], FP8, name=f"vv8_{g}", tag=f"vv8_{g}")
               for g in range(NP2)]
        at8 = [pmisc.tile([P, 2, 512], FP8, name=f"at8_{g}")
               for g in range(CP)]
        wf8 = {nm: [pw.tile([P, 2, C], FP8, name=f"wf8_{nm}_{g}",
                            tag=f"wf8_{nm}_{g}") for g in range(CP)]
               for nm in ("wq", "wk", "wv", "wo")}
        w16 = {nm: [pw.tile([P, C], BF16, name=f"w16_{nm}_{ct}",
                            tag=f"w16_{nm}_{ct}") for ct in range(CT)]
               for nm in ("wq", "wk", "wv", "wo")}
        resb = [pres.tile([P, C], F32, name=f"resb_{i}", tag=f"resb_{i}")
                for i in range(QTILES)]

        with ExitStack() as dphase:
            psw = dphase.enter_context(
                tc.tile_pool(name="psw", bufs=1, space="PSUM"))
            psb = dphase.enter_context(
                tc.tile_pool(name="psb", bufs=2, space="PSUM"))
            pskq = dphase.enter_context(
                tc.tile_pool(name="pskq", bufs=2, space="PSUM"))
            psv = dphase.enter_context(
                tc.tile_pool(name="psv", bufs=2, space="PSUM"))
            ptmp = dphase.enter_context(tc.tile_pool(name="ptmp", bufs=2))

            # Warm-keeper: tiny fp8 matmuls keep the PE clock ungated while
            # DMA/DVE run the preamble.
            warm8 = pmisc.tile([P, P], FP8, name="warm8")
            nc.vector.memset(warm8, 0.5)

            def keep_warm(n):
                for _ in range(n):
                    wps = psw.tile([P, P], F32, name="wps", tag="wps")
                    nc.tensor.matmul(wps, lhsT=warm8, rhs=warm8,
                                     start=True, stop=True)

            # ---- Phase A: DMA x^T in; bn_stats as chunks land ----
            bnst = [pmisc.tile([P, 8, 6], F32, name=f"bnst{ct}")
                    for ct in range(CT)]
            for ch in range(4):
                c0, c1 = ch * 1024, (ch + 1) * 1024
                for g in range(CP):
                    nc.sync.dma_start(out=xt8[g][:, :, c0:c1],
                                      in_=xt8_d[g][:, :, c0:c1])
                for g in range(CP):
                    for i in range(2):
                        nc.vector.bn_stats(
                            out=bnst[2 * g + i][:, 2 * ch, :],
                            in_=xt8[g][:, i, c0:c0 + 512])
                        nc.vector.bn_stats(
                            out=bnst[2 * g + i][:, 2 * ch + 1, :],
                            in_=xt8[g][:, i, c0 + 512:c1])
                keep_warm(2)

            # DMA weights + query-quarter x^T + residuals
            nc.sync.dma_start(out=xq8, in_=xq8_d[:])
            for nm in ("wk", "wq", "wv", "wo"):
                for ct in range(CT):
                    nc.sync.dma_start(
                        out=w16[nm][ct],
                        in_=w16_d[nm][ct * P:(ct + 1) * P, :])

            # ---- Phase B: stats -> a, b ----
            a4, aS4, b4, b16 = [], [], [], []
            for ct in range(CT):
                mv = ptmp.tile([P, 2], F32, name="mv", tag="mv")
                nc.vector.bn_aggr(out=mv, in_=bnst[ct])
                me2 = ptmp.tile([P, 2], F32, name="me2", tag="me2")
                nc.vector.tensor_copy(me2[:, 0:1], mv[:, 0:1])
                nc.vector.tensor_mul(me2[:, 1:2], mv[:, 0:1], mv[:, 0:1])
                nc.vector.tensor_add(me2[:, 1:2], me2[:, 1:2], mv[:, 1:2])
                grp_ps = psb.tile([8, 2], F32, name="grp_ps", tag="bias")
                nc.tensor.matmul(grp_ps, lhsT=gind, rhs=me2,
                                 start=True, stop=True)
                grp = ptmp.tile([8, 2], F32, name="grp", tag="grp")
                nc.vector.tensor_scalar_mul(grp, grp_ps, 1.0 / GS)
                var = ptmp.tile([8, 1], F32, name="var", tag="var")
                nc.vector.tensor_mul(var, grp[:, 0:1], grp[:, 0:1])
                nc.vector.tensor_sub(var, grp[:, 1:2], var)
                nc.vector.tensor_scalar_add(var, var, EPS)
                rstd = ptmp.tile([8, 1], F32, name="rstd", tag="rstd")
                nc.vector.reciprocal(rstd, var)
                nc.scalar.sqrt(rstd, rstd)
                mr = ptmp.tile([8, 2], F32, name="mr", tag="mr")
                nc.vector.tensor_copy(mr[:, 0:1], grp[:, 0:1])
                nc.vector.tensor_copy(mr[:, 1:2], rstd)
                keep_warm(2)
                mch_ps = psb.tile([P, 2], F32, name="mch_ps", tag="bias")
                nc.tensor.matmul(mch_ps, lhsT=gindt, rhs=mr,
                                 start=True, stop=True)
                mch = ptmp.tile([P, 2], F32, name="mch", tag="mch")
                nc.vector.tensor_copy(mch, mch_ps)
                a_t = pmisc.tile([P, 1], F32, name=f"a4_{ct}")
                nc.vector.tensor_mul(a_t, gamma4[ct], mch[:, 1:2])
                a4.append(a_t)
                aS_t = pmisc.tile([P, 1], F32, name=f"aS4_{ct}")
                nc.vector.tensor_scalar_mul(aS_t, a_t, S)
                aS4.append(aS_t)
                b_t = pmisc.tile([P, 1], F32, name=f"b4_{ct}")
                nc.vector.tensor_mul(b_t, mch[:, 0:1], a_t)
                nc.vector.tensor_sub(b_t, beta4[ct], b_t)
                b4.append(b_t)
                b16_t = pmisc.tile([P, 1], BF16, name=f"b16_{ct}")
                nc.vector.tensor_copy(b16_t, b_t)
                b16.append(b16_t)

            # ---- Phase C: fold weights to fp8 pairs + biases ----
            for nm, scales in (("wk", aS4), ("wq", aS4), ("wv", aS4),
                               ("wo", None)):
                for ct in range(CT):
                    g, i = divmod(ct, 2)
                    if scales is None:
                        nc.vector.tensor_scalar_mul(
                            wf8[nm][g][:, i, :], w16[nm][ct], S)
                    else:
                        nc.vector.tensor_scalar_mul(
                            wf8[nm][g][:, i, :], w16[nm][ct], scales[ct])
            keep_warm(2)

            def bias_gemv(nm, lhs16):
                """[1, C] = sum_ct lhs16[ct]^T @ w16[nm][ct]  (bf16)."""
                bps = psb.tile([1, C], F32, name=f"bps_{nm}", tag="bias")
                for ct in range(CT):
                    nc.tensor.matmul(bps, lhsT=lhs16[ct], rhs=w16[nm][ct],
                                     start=(ct == 0), stop=(ct == CT - 1))
                braw = ptmp.tile([1, C], F32, name=f"braw_{nm}",
                                 tag="braw")
                nc.sync.dma_start(out=braw, in_=b_d["b" + nm[1:]][:])
                bsb = pmisc.tile([1, C], F32, name=f"bias_{nm}")
                nc.vector.tensor_add(bsb, bps, braw)
                return bsb

            def per_partition(bsb, nm):
                out = []
                for co in range(CT):
                    pps = psb.tile([P, 1], F32, name=f"pps_{nm}{co}",
                                   tag="bias")
                    nc.tensor.matmul(pps,
                                     lhsT=bsb[0:1, co * P:(co + 1) * P],
                                     rhs=one11, start=True, stop=True)
                    bp = pmisc.tile([P, 1], F32, name=f"bp_{nm}{co}")
                    nc.vector.tensor_copy(bp, pps)
                    out.append(bp)
                return out

            bk_sb = bias_gemv("wk", b16)
            bk4 = per_partition(bk_sb, "k")
            bq_sb = bias_gemv("wq", b16)
            bq4 = per_partition(bq_sb, "q")
            bv_sb = bias_gemv("wv", b16)
            bv4 = per_partition(bv_sb, "v")
            bv16 = []
            for ct in range(CT):
                t = pmisc.tile([P, 1], BF16, name=f"bv16_{ct}")
                nc.vector.tensor_copy(t, bv4[ct])
                bv16.append(t)
            bo_sb = bias_gemv("wo", bv16)  # wo^T bv_full + bo
            keep_warm(2)

            bo_b = pmisc.tile([P, C], F32, name="bo_b")
            nc.gpsimd.partition_broadcast(bo_b, bo_sb)
            for i in range(QTILES):
                rraw = ptmp.tile([P, C], F32, name="rraw", tag="rraw")
                nc.sync.dma_start(out=rraw,
                                  in_=x_res_d[i * P:(i + 1) * P, :])
                nc.vector.tensor_add(resb[i], rraw, bo_b)

            # ---- Phase D: projections (fp8 DoubleRow) ----
            def kq_proj(dst_pairs, wname, bias4, rhs_of):
                """dst[co] tile chunk = (sum_g wf8^T x) / S + bias."""
                for ch in range(rhs_of[1]):
                    for co in range(CT):
                        ps = pskq.tile([P, 512], F32, name="kqps", tag="kq")
                        for g in range(CP):
                            nc.tensor.matmul(
                                ps,
                                lhsT=wf8[wname][g][:, :, co * P:(co + 1) * P],
                                rhs=rhs_of[0](g, ch),
                                start=(g == 0), stop=(g == CP - 1),
                                perf_mode=DR)
                        og, oi = divmod(co, 2)
                        nc.vector.tensor_scalar(
                            dst_pairs[og][:, oi, ch * 512:(ch + 1) * 512],
                            ps, 1.0 / S, bias4[co], ALU.mult, ALU.add)

            kq_proj(kt8, "wk", bk4,
                    (lambda g, ch: xt8[g][:, :, ch * 512:(ch + 1) * 512], 8))
            kq_proj(qt8, "wq", bq4,
                    (lambda g, ch: xq8[:, 2 * g:2 * g + 2,
                                       ch * 512:(ch + 1) * 512], QCH))

            for nt in range(NT):
                ps = psv.tile([P, 512], F32, name="vps", tag="v")
                for g in range(CP):
                    nc.tensor.matmul(
                        ps, lhsT=xt8[g][:, :, nt * P:(nt + 1) * P],
                        rhs=wf8["wv"][g], start=(g == 0), stop=(g == CP - 1),
                        perf_mode=DR)
                og, oi = divmod(nt, 2)
                nc.scalar.mul(vv8[og][:, oi, :], ps, 1.0 / S)

        # ---- Phase E: attention + output projection ----
        with tc.tile_pool(name="pss", bufs=2, space="PSUM") as pss, \
             tc.tile_pool(name="psat", bufs=1, space="PSUM") as psat, \
             tc.tile_pool(name="psr", bufs=1, space="PSUM") as psr, \
             tc.tile_pool(name="pso", bufs=1, space="PSUM") as pso, \
             tc.tile_pool(name="pe", bufs=3) as pe, \
             tc.tile_pool(name="pef", bufs=2) as pef:
            for qc in range(QCH):
                at_ps = [psat.tile([P, 512], F32, name=f"at{co}",
                                   tag=f"at{co}") for co in range(CT)]
                rows_ps = psr.tile([32, 512], F32, name="rows", tag="rows")

                def attn_v(g, probs):
                    for co in range(CT):
                        nc.tensor.matmul(
                            at_ps[co],
                            lhsT=vv8[g][:, :, co * P:(co + 1) * P],
                            rhs=probs,
                            start=(g == 0), stop=(g == NP2 - 1),
                            perf_mode=DR)
                    nc.tensor.matmul(rows_ps, lhsT=ones32_8, rhs=probs,
                                     start=(g == 0), stop=(g == NP2 - 1),
                                     perf_mode=DR)

                prev = None
                for g in range(NP2):
                    scs = []
                    for j in range(2):
                        kt_i = 2 * g + j
                        sc = pss.tile([P, 512], F32, name="sc", tag="sc")
                        for c in range(CP):
                            nc.tensor.matmul(
                                sc,
                                lhsT=kt8[c][:, :, kt_i * P:(kt_i + 1) * P],
                                rhs=qt8[c][:, :, qc * 512:(qc + 1) * 512],
                                start=(c == 0), stop=(c == CP - 1),
                                perf_mode=DR)
                        scs.append(sc)
                    if prev is not None:
                        attn_v(g - 1, prev)
                    probs = pe.tile([P, 2, 512], FP8, name="probs",
                                    tag="probs")
                    for j in range(2):
                        nc.scalar.activation(probs[:, j, :], scs[j], AF.Exp,
                                             bias=ebias, scale=ISQ)
                    prev = probs
                attn_v(NP2 - 1, prev)

                recipS = pe.tile([1, 512], F32, name="recipS", tag="recipS")
                nc.vector.reciprocal(recipS, rows_ps[0:1, :])
                nc.vector.tensor_scalar_mul(recipS, recipS, S)
                rSb = pe.tile([P, 512], F32, name="rSb", tag="rSb")
                nc.gpsimd.partition_broadcast(rSb, recipS)
                for co in range(CT):
                    og, oi = divmod(co, 2)
                    nc.vector.tensor_mul(at8[og][:, oi, :], at_ps[co], rSb)

                for qt in range(4):
                    ops = pso.tile([P, C], F32, name="ops", tag="ops")
                    for g in range(CP):
                        nc.tensor.matmul(
                            ops, lhsT=at8[g][:, :, qt * P:(qt + 1) * P],
                            rhs=wf8["wo"][g], start=(g == 0),
                            stop=(g == CP - 1), perf_mode=DR)
                    fin = pef.tile([P, C], F32, name="fin", tag="fin")
                    nc.scalar.mul(fin, ops, 1.0 / (S * S))
                    fin2 = pef.tile([P, C], F32, name="fin2", tag="fin2")
                    nc.vector.tensor_add(fin2, fin, resb[qc * 4 + qt])
                    r0 = (qc * 4 + qt) * P
                    nc.sync.dma_start(out=out_d[r0:r0 + P, :], in_=fin2)

    nc.compile()
    return nc


def _consts():
    gind = np.zeros((P, 8), dtype=np.float32)
    for p in range(P):
        gind[p, p // GS] = 1.0
    gindt = np.ascontiguousarray(gind.T)
    return gind, gindt


def _make_in_maps(inputs):
    import ml_dtypes
    FP8NP = ml_dtypes.float8_e4m3
    x = np.ascontiguousarray(np.asarray(inputs["inputs"], dtype=np.float32))
    xf = x.reshape(B, N, C)
    gind, gindt = _consts()
    shared = {
        "gamma": np.ascontiguousarray(np.asarray(inputs["gn_gamma"], np.float32)),
        "beta": np.ascontiguousarray(np.asarray(inputs["gn_beta"], np.float32)),
        "gind": gind, "gindt": gindt,
        "one11": np.ones((1, 1), np.float32),
    }
    for nm in ("wq", "wk", "wv", "wo"):
        shared[nm] = np.ascontiguousarray(
            np.asarray(inputs[nm], np.float32).astype(ml_dtypes.bfloat16))
    for nm in ("bq", "bk", "bv", "bo"):
        shared[nm] = np.ascontiguousarray(np.asarray(inputs[nm], np.float32))

    # x^T fp8 channel-pair tiles: xt8p[g][p, i, n] = x^T[g*256 + i*128 + p, n]
    xt_pairs = []
    for b in range(B):
        xT = np.ascontiguousarray(xf[b].T).astype(FP8NP)      # [512, 4096]
        xp = xT.reshape(CP, 2, P, N).transpose(0, 2, 1, 3)    # [2][128, 2, N]
        xt_pairs.append([np.ascontiguousarray(xp[g]) for g in range(CP)])

    in_maps = []
    for core in range(NCORES):
        b, qq = divmod(core, 4)
        m = dict(shared)
        for g in range(CP):
            m[f"xt8p{g}"] = xt_pairs[b][g]
        # query-quarter columns, stacked pairs: [128, 2*CP, QS]
        xq = np.concatenate(
            [xt_pairs[b][g][:, :, qq * QS:(qq + 1) * QS] for g in range(CP)],
            axis=1)
        m["xq8p"] = np.ascontiguousarray(xq)
        m["x_res"] = np.ascontiguousarray(xf[b, qq * QS:(qq + 1) * QS, :])
        in_maps.append(m)
    return in_maps


def _assemble(results):
    out = np.empty((B, N, C), dtype=np.float32)
    for core in range(NCORES):
        b, qq = divmod(core, 4)
        out[b, qq * QS:(qq + 1) * QS, :] = results[core]["out"]
    return out.reshape(B, HH, WW, C)


def kernel(**inputs):
    global _NC_CACHE
    if _NC_CACHE is None:
        _NC_CACHE = _build()
    in_maps = _make_in_maps(inputs)
    res = run_bass_kernel_spmd(_NC_CACHE, in_maps, list(range(NCORES)))
    return _assemble(res.results)


def _install_ntff_shim():
    """The agent image's antenv lacks axon_hooks; provide it so
    run_bass_kernel_spmd(trace=True) can NTFF-profile through axon."""
    import types
    import antenv
    if "antenv.axon_hooks" in sys.modules:
        return
    mod = types.ModuleType("antenv.axon_hooks")
    mod._hook = None

    def set_axon_ntff_profile_hook(h):
        mod._hook = h

    def get_axon_ntff_profile_hook():
        return mod._hook

    mod.set_axon_ntff_profile_hook = set_axon_ntff_profile_hook
    mod.get_axon_ntff_profile_hook = get_axon_ntff_profile_hook
    sys.modules["antenv.axon_hooks"] = mod
    antenv.axon_hooks = mod
    sys.path.insert(0, "/root/.axon_site")
    from trn_agent_boot.trn_boot import _ntff_profile_via_ctypes
    hook = _ntff_profile_via_ctypes("/opt/axon/libaxon_pjrt.so")
    set_axon_ntff_profile_hook(hook)


def run_traced(inputs, trace_kwargs=None):
    """Traced run for profiling: returns (BassKernelResults, tmpdir)."""
    global _NC_CACHE
    if _NC_CACHE is None:
        _NC_CACHE = _build()
    import tempfile
    _install_ntff_shim()
    in_maps = _make_in_maps(inputs)
    tmpdir = tempfile.mkdtemp(prefix="trace_")
    res = run_bass_kernel_spmd(_NC_CACHE, in_maps, list(range(NCORES)),
                               trace=True, tmpdir=tmpdir,
                               trace_kwargs=trace_kwargs or {})
    return res, tmpdir
